# revision 9
# baseline (speedup 1.0000x reference)
"""Trainium2 Bass kernel for nn_DiffPoolModel (gnn_message_passing).

Strategy (data-parallel over graphs, 32 graphs per core x 8 cores):
  Every graph has exactly 128 nodes (= SBUF partition count) and intra-graph
  edges, so the sparse GNN becomes batched dense linear algebra:
    - Dense adjacency A^T per graph built on the PE: one-hot edge matrices
      via tensor_scalar(is_equal) compares (DVE/GPSIMD) + bf16 matmul
      accumulation over 16 edge chunks.  Exact (counts in fp32 PSUM).
    - SAGE layers: transforms as lhsT(feature-major activation) @ W(moving),
      producing node-major outputs; mean-aggregations as
      lhsT(node-major activation) @ A^T(moving), producing feature-major
      outputs; 1/max(cnt,1) normalization folded in node-major (per
      partition) after the neighbor transform.
    - Level 2 (20-node pooled graphs) packs 4 graphs per 80-partition tile
      with block-diagonal adjacency.
    - Losses reduced to per-graph scalars via accum_out columns + one-hot
      block-sum matmuls; final scalar math on tiny [1,32]/[4,8] tiles.

kernel(**inputs) takes FULL inputs (x [32768,64], edge_index [2,524288],
edge_weight [524288], node_graph_index [32768], params pytree) and returns
(logits [256,2], cut_loss, orth_loss) matching reference.py.
"""

import os
from contextlib import ExitStack

import numpy as np
import ml_dtypes

import concourse.bass as bass
import concourse.bacc as bacc
import concourse.tile as tile
from concourse import mybir
from concourse import bass_utils

# ---------------------------------------------------------------- constants
NUM_GRAPHS = 256
M = 128              # nodes per graph
EPG = 2048           # edges per graph
NCHUNK = EPG // 128  # 16 edge chunks of 128
IN_FEAT = 64
F = 128              # hidden features (both levels)
K1 = 20              # level-1 clusters
K2 = 5               # level-2 clusters
NCORES = 8
GPC = NUM_GRAPHS // NCORES   # 32 graphs per core
L2PACK = 4                   # graphs per level-2 tile (4*20 = 80 partitions)
NL2 = GPC // L2PACK          # 8 level-2 tiles per core
EPS = 1e-9

BF = mybir.dt.float16  # fp16: same PE/DVE rates as bf16, 8x mantissa
F32 = mybir.dt.float32

bf16 = np.float16

AluOp = mybir.AluOpType
Act = mybir.ActivationFunctionType


# ---------------------------------------------------------------- host prep
def _edge_layout(idx, g0, g1):
    """[128, (g1-g0)*16] layout: [p, (g-g0)*16+k] = value of edge (g, k*128+p)."""
    out = np.empty((128, (g1 - g0) * NCHUNK), np.float32)
    for g in range(g0, g1):
        blk = idx[g * EPG:(g + 1) * EPG].reshape(NCHUNK, 128).T
        out[:, (g - g0) * NCHUNK:(g - g0 + 1) * NCHUNK] = blk
    return np.ascontiguousarray(out)


def shard_inputs(x, edge_index, edge_weight, params, core, gpc=GPC):
    """Build the per-core input map (numpy arrays keyed by DRAM tensor name)."""
    g0, g1 = core * gpc, (core + 1) * gpc
    n0, n1 = g0 * M, g1 * M

    xs = np.asarray(x[n0:n1], np.float32)                     # [gpc*128, 64]
    # node-major batched: [128, gpc*64], graph g block at cols [64g, 64g+64)
    x_nm = np.ascontiguousarray(
        xs.reshape(gpc, M, IN_FEAT).transpose(1, 0, 2).reshape(M, gpc * IN_FEAT)
    )
    # feature-major batched with ones row: [65, gpc*128]
    xT = xs.reshape(gpc, M, IN_FEAT).transpose(2, 0, 1).reshape(IN_FEAT, gpc * M)
    xT_aug = np.concatenate([xT, np.ones((1, gpc * M), np.float32)], 0)

    row = np.asarray(edge_index[0], np.int64)
    col = np.asarray(edge_index[1], np.int64)
    ew = np.asarray(edge_weight, np.float32)
    e0, e1 = g0 * EPG, g1 * EPG
    rloc = (row[e0:e1] - (row[e0:e1] // M) * M).astype(np.float32)
    clocal = (col[e0:e1] - (col[e0:e1] // M) * M).astype(np.float32)

    erows = _edge_layout(rloc, 0, gpc)
    ecols = _edge_layout(clocal, 0, gpc)
    ews = _edge_layout(ew[e0:e1], 0, gpc)

    return {
        "x_nm": x_nm.astype(bf16),
        "xT_aug": xT_aug.astype(bf16),
        "erows": erows,
        "ecols": ecols,
        "ews": ews,
    }


def pack_params(params):
    """Shared (core-independent) weight tensors."""
    lv = params["levels"]

    def wpair(layer):
        ws, wn, b = layer
        return (np.asarray(ws, np.float32), np.asarray(wn, np.float32),
                np.asarray(b, np.float32))

    out = {}
    for li, lvl in enumerate(lv):
        for net, key in (("f", "feat"), ("a", "assign")):
            for lj in range(2):
                ws, wn, b = wpair(lvl[key][lj])
                if li == 0 and lj == 0:
                    # level-1 layer-0: fold bias as extra contraction row
                    # (lhsT = xT_aug with ones row)
                    ws = np.concatenate([ws, b[None, :]], 0)
                out[f"w_{li}_{net}{lj}_s"] = ws.astype(bf16)
                out[f"w_{li}_{net}{lj}_n"] = wn.astype(bf16)
                out[f"b_{li}_{net}{lj}"] = b[None, :].astype(bf16)
        out[f"poolb_{li}"] = np.asarray(lvl["pool_bias"], np.float32)[:, None]
    mw = np.asarray(params["mlp_w"], np.float32)      # [256, 2]
    out["mlp_w1"] = mw[:F]
    out["mlp_w2"] = mw[F:]
    out["mlp_b"] = np.asarray(params["mlp_b"], np.float32)[None, :]
    # level-2 packing: 4 graphs per 128-partition tile, graph j occupies
    # partitions [32j, 32j+20) (starts must be multiples of 32).
    bmask = np.zeros((128, L2PACK * K2), np.float32)
    for j in range(L2PACK):
        bmask[32 * j:32 * j + K1, K2 * j:K2 * (j + 1)] = 1.0
    out["bmask"] = bmask.astype(bf16)
    # block-sum one-hots
    blk5 = np.zeros((L2PACK * K2, L2PACK), np.float32)
    for j in range(L2PACK):
        blk5[K2 * j:K2 * (j + 1), j] = 1.0
    out["blk5"] = blk5.astype(np.float32)
    blk20 = np.zeros((128, L2PACK), np.float32)
    for j in range(L2PACK):
        blk20[32 * j:32 * j + K1, j] = 1.0
    out["blk20"] = blk20.astype(np.float32)
    return out


# ------------------------------------------------------------- the program
def build_program(gpc=GPC, debug=False):
    nc = bacc.Bacc("TRN2", target_bir_lowering=False, debug=False)
    nl2 = gpc // L2PACK
    assert gpc % L2PACK == 0

    # ---- DRAM io
    dram = {}

    def din(name, shape, dt):
        dram[name] = nc.dram_tensor(name, shape, dt, kind="ExternalInput").ap()

    def dout(name, shape, dt):
        dram[name] = nc.dram_tensor(name, shape, dt, kind="ExternalOutput").ap()

    din("x_nm", [M, gpc * IN_FEAT], BF)
    din("xT_aug", [IN_FEAT + 1, gpc * M], BF)
    din("erows", [128, gpc * NCHUNK], F32)
    din("ecols", [128, gpc * NCHUNK], F32)
    din("ews", [128, gpc * NCHUNK], F32)
    for li, din_f in ((0, IN_FEAT), (1, F)):
        for net in "fa":
            for lj in range(2):
                d_in = din_f if lj == 0 else F
                d_out = F if (lj == 0 or net == "f") else (K1 if li == 0 else K2)
                d_in_s = d_in + 1 if (li == 0 and lj == 0) else d_in
                din(f"w_{li}_{net}{lj}_s", [d_in_s, d_out], BF)
                din(f"w_{li}_{net}{lj}_n", [d_in, d_out], BF)
                din(f"b_{li}_{net}{lj}", [1, d_out], BF)
        din(f"poolb_{li}", [F, 1], F32)
    din("mlp_w1", [F, 2], F32)
    din("mlp_w2", [F, 2], F32)
    din("mlp_b", [1, 2], F32)
    din("bmask", [128, L2PACK * K2], BF)
    din("blk5", [L2PACK * K2, L2PACK], F32)
    din("blk20", [128, L2PACK], F32)

    dout("logitsT", [2, gpc], F32)
    dout("losses", [1, 2], F32)
    if debug:
        dout("dbg_h1", [M, F], F32)        # graph 0 level-1 h1 (node-major)
        dout("dbg_S", [M, K1], F32)        # graph 0 S
        dout("dbg_pxT", [F, K1], F32)      # graph 0 pooled_x^T
        dout("dbg_SAS", [K1, K1], F32)
        dout("dbg_scal", [128, 4 * gpc], F32)

    with tile.TileContext(nc) as tc:
        _build_tile_program(tc, dram, gpc, nl2, debug)
    nc.compile()
    return nc, dram


def _build_tile_program(tc, dram, gpc, nl2, debug):
    nc = tc.nc
    ctx = ExitStack()

    singles = ctx.enter_context(tc.tile_pool(name="singles", bufs=1))
    onehot = ctx.enter_context(tc.tile_pool(name="onehot", bufs=3))
    ev = ctx.enter_context(tc.tile_pool(name="ev", bufs=3))
    nm = ctx.enter_context(tc.tile_pool(name="nm", bufs=3))
    fm = ctx.enter_context(tc.tile_pool(name="fm", bufs=3))
    small = ctx.enter_context(tc.tile_pool(name="small", bufs=4))
    # PSUM budget: 8 banks total.  ps2 tags: at(2) + agg(2) = 4 banks;
    # ps1 tags: tself(1) + tnei(1) + trp(1) + small(1) = 4 banks.
    ps2 = ctx.enter_context(tc.tile_pool(name="ps2", bufs=2, space="PSUM"))
    ps1 = ctx.enter_context(tc.tile_pool(name="ps1", bufs=1, space="PSUM"))

    # ---------------- persistent constants & inputs
    iota_bf = singles.tile([128, 128], BF)
    nc.gpsimd.iota(iota_bf[:], pattern=[[1, 128]], base=0, channel_multiplier=0,
                   allow_small_or_imprecise_dtypes=True)
    ident = singles.tile([128, 128], BF)
    nc.gpsimd.memset(ident[:], 0.0)
    nc.gpsimd.affine_select(out=ident[:], in_=ident[:],
                            compare_op=AluOp.not_equal, fill=1.0,
                            base=0, pattern=[[-1, 128]], channel_multiplier=1)
    ones_col = singles.tile([128, 1], BF)
    nc.vector.memset(ones_col[:], 1.0)
    ones_row = singles.tile([1, 128], BF)
    nc.vector.memset(ones_row[:], 1.0)
    ones_col_f = singles.tile([128, 1], F32)
    nc.vector.memset(ones_col_f[:], 1.0)

    x_nm = singles.tile([M, gpc * IN_FEAT], BF)
    nc.sync.dma_start(out=x_nm[:], in_=dram["x_nm"][:, :])
    xT_aug = singles.tile([IN_FEAT + 1, gpc * M], BF)
    nc.sync.dma_start(out=xT_aug[:], in_=dram["xT_aug"][:, :])
    erows = singles.tile([128, gpc * NCHUNK], F32)
    nc.sync.dma_start(out=erows[:], in_=dram["erows"][:, :])
    ecols = singles.tile([128, gpc * NCHUNK], F32)
    nc.sync.dma_start(out=ecols[:], in_=dram["ecols"][:, :])
    ews = singles.tile([128, gpc * NCHUNK], F32)
    nc.sync.dma_start(out=ews[:], in_=dram["ews"][:, :])

    W = {}
    for name, ap in dram.items():
        if name.startswith(("w_", "b_")):
            t = singles.tile(list(ap.shape), BF, tag=f"W_{name}")
            nc.sync.dma_start(out=t[:], in_=ap[:, :])
            W[name] = t
    poolb = {}
    for li in range(2):
        t = singles.tile([F, 1], F32, tag=f"poolb_{li}")
        nc.sync.dma_start(out=t[:], in_=dram[f"poolb_{li}"][:, :])
        poolb[li] = t
    mlp_w1 = singles.tile([F, 2], F32)
    nc.sync.dma_start(out=mlp_w1[:], in_=dram["mlp_w1"][:, :])
    mlp_w2 = singles.tile([F, 2], F32)
    nc.sync.dma_start(out=mlp_w2[:], in_=dram["mlp_w2"][:, :])
    mlp_b = singles.tile([1, 2], F32)
    nc.sync.dma_start(out=mlp_b[:], in_=dram["mlp_b"][:, :])
    bmask = singles.tile([128, L2PACK * K2], BF)
    nc.sync.dma_start(out=bmask[:], in_=dram["bmask"][:, :])
    blk5 = singles.tile([L2PACK * K2, L2PACK], F32)
    nc.sync.dma_start(out=blk5[:], in_=dram["blk5"][:, :])
    blk20 = singles.tile([128, L2PACK], F32)
    nc.sync.dma_start(out=blk20[:], in_=dram["blk20"][:, :])

    # persistent accumulators / stashes
    GH1 = singles.tile([F, gpc], F32)
    GH2 = singles.tile([F, gpc], F32)
    # level-1 per-graph scalar columns: slots [num | den | q | t] x gpc
    SCAL1 = singles.tile([128, 4 * gpc], F32)
    nc.vector.memset(SCAL1[:], 0.0)
    # level-2: 20-row slots (3 types x nl2), 80-row slots (1 x nl2)
    SCAL2a = singles.tile([128, 3 * nl2], F32)
    nc.vector.memset(SCAL2a[:], 0.0)
    SCAL2b = singles.tile([128, nl2], F32)
    nc.vector.memset(SCAL2b[:], 0.0)
    # level-2 batched tiles (graph j of a tile on partitions [32j, 32j+20))
    PXT = singles.tile([F, gpc * 32], BF)            # feature-major pooled x
    nc.vector.memset(PXT[:], 0.0)
    PXN = singles.tile([128, nl2 * F], BF)           # node-major pooled x
    nc.vector.memset(PXN[:], 0.0)
    A2T = singles.tile([128, nl2 * 128], BF)
    nc.vector.memset(A2T[:], 0.0)

    # ---------------------------------------------------------------- level 1
    def sage_combine(tself_ps, tnei_ps, inv_ap, relu, dims, nm_out_dt=BF):
        """node-major out = maybe_relu(tself + inv*(tnei)) ; returns nm tile."""
        p, f = dims
        ts_sb = ev.tile([p, f], F32, tag="ts_sb")
        nc.scalar.copy(out=ts_sb[:], in_=tself_ps[:])
        w_sb = nm.tile([p, f], F32, tag="w_sb")
        nc.vector.scalar_tensor_tensor(
            out=w_sb[:], in0=tnei_ps[:], scalar=inv_ap, in1=ts_sb[:],
            op0=AluOp.mult, op1=AluOp.add)
        h_nm = nm.tile([p, f], nm_out_dt, tag="h_nm")
        if relu:
            nc.vector.tensor_scalar(out=h_nm[:], in0=w_sb[:], scalar1=0.0,
                                    scalar2=None, op0=AluOp.max)
        else:
            nc.vector.tensor_copy(out=h_nm[:], in_=w_sb[:])
        return h_nm

    for g in range(gpc):
        # ---- A^T build
        at_ps = ps2.tile([128, 128], F32, tag="at")
        for k in range(NCHUNK):
            c = g * NCHUNK + k
            rk = onehot.tile([128, 128], BF, tag="rk")
            ck = onehot.tile([128, 128], BF, tag="ck")
            nc.vector.tensor_scalar(out=rk[:], in0=iota_bf[:],
                                    scalar1=erows[:, c:c + 1], scalar2=None,
                                    op0=AluOp.is_equal)
            # fold edge weight into col side
            nc.vector.tensor_scalar(out=ck[:], in0=iota_bf[:],
                                    scalar1=ecols[:, c:c + 1],
                                    scalar2=ews[:, c:c + 1],
                                    op0=AluOp.is_equal, op1=AluOp.mult)
            nc.tensor.matmul(at_ps[:], lhsT=ck[:], rhs=rk[:],
                             start=(k == 0), stop=(k == NCHUNK - 1))
        at_bf = ev.tile([128, 128], BF, tag="at_bf")
        nc.scalar.copy(out=at_bf[:], in_=at_ps[:])

        # cnt = deg = rowsum(A) ; inv = 1/max(cnt, 1)
        cnt_ps = ps1.tile([128, 1], F32, tag="small")
        nc.tensor.matmul(cnt_ps[:], lhsT=at_bf[:], rhs=ones_col[:],
                         start=True, stop=True)
        deg = small.tile([128, 1], F32, tag="deg")
        nc.vector.tensor_copy(out=deg[:], in_=cnt_ps[:])
        mx = small.tile([128, 1], F32, tag="mx")
        nc.vector.tensor_scalar(out=mx[:], in0=cnt_ps[:], scalar1=1.0,
                                scalar2=None, op0=AluOp.max)
        inv = small.tile([128, 1], F32, tag="inv")
        nc.vector.reciprocal(out=inv[:], in_=mx[:])

        xT_g = xT_aug[:, g * M:(g + 1) * M]          # [65, 128] fm with ones row
        x_nm_g = x_nm[:, g * IN_FEAT:(g + 1) * IN_FEAT]

        # ---- shared aggregation of x: m1T = (A x)^T  [64, 128]
        m1_ps = ps2.tile([IN_FEAT, M], F32, tag="agg")
        nc.tensor.matmul(m1_ps[:], lhsT=x_nm_g, rhs=at_bf[:], start=True, stop=True)
        m1T = fm.tile([IN_FEAT, M], BF, tag="m1T")
        nc.scalar.copy(out=m1T[:], in_=m1_ps[:])

        def transform(inT, in_mT, wkey, dims, bias=True):
            """psum pair (tself+bias, tnei) for one SAGE layer.
            bias=False: bias already folded via augmented lhsT/weight row."""
            p, dout_ = dims
            tself = ps1.tile([p, dout_], F32, tag="tself")
            nc.tensor.matmul(tself[:], lhsT=inT, rhs=W[wkey + "_s"][:],
                             start=True, stop=not bias)
            if bias:
                nc.tensor.matmul(tself[:], lhsT=ones_row[:, :p],
                                 rhs=W["b" + wkey[1:]][:],
                                 start=False, stop=True)
            tnei = ps1.tile([p, dout_], F32, tag="tnei")
            nc.tensor.matmul(tnei[:], lhsT=in_mT, rhs=W[wkey + "_n"][:],
                             start=True, stop=True)
            return tself, tnei

        # ---- feat layer 1: h1 = relu(x@Ws + b + inv*(m1@Wn))
        ts, tn = transform(xT_g, m1T[:], "w_0_f0", (M, F), bias=False)
        h1_nm = sage_combine(ts, tn, inv[:], True, (M, F))
        # transpose for next layer lhsT
        h1T_ps = ps1.tile([F, M], BF, tag="trp")
        nc.tensor.transpose(out=h1T_ps[:], in_=h1_nm[:], identity=ident[:])
        h1T = fm.tile([F, M], BF, tag="h1T")
        nc.scalar.copy(out=h1T[:], in_=h1T_ps[:])
        # m2T = (A h1)^T
        m2_ps = ps2.tile([F, M], F32, tag="agg")
        nc.tensor.matmul(m2_ps[:], lhsT=h1_nm[:], rhs=at_bf[:], start=True, stop=True)
        m2T = fm.tile([F, M], BF, tag="m2T")
        nc.scalar.copy(out=m2T[:], in_=m2_ps[:])
        # feat layer 2 (no relu): h
        ts, tn = transform(h1T[:], m2T[:], "w_0_f1", (M, F))
        h_nm = sage_combine(ts, tn, inv[:], False, (M, F))

        # ---- assign layer 1
        ts, tn = transform(xT_g, m1T[:], "w_0_a0", (M, F), bias=False)
        a1_nm = sage_combine(ts, tn, inv[:], True, (M, F))
        a1T_ps = ps1.tile([F, M], BF, tag="trp")
        nc.tensor.transpose(out=a1T_ps[:], in_=a1_nm[:], identity=ident[:])
        a1T = fm.tile([F, M], BF, tag="a1T")
        nc.scalar.copy(out=a1T[:], in_=a1T_ps[:])
        ma_ps = ps2.tile([F, M], F32, tag="agg")
        nc.tensor.matmul(ma_ps[:], lhsT=a1_nm[:], rhs=at_bf[:], start=True, stop=True)
        maT = fm.tile([F, M], BF, tag="m2T")
        nc.scalar.copy(out=maT[:], in_=ma_ps[:])
        # assign layer 2 -> a2 [128, 20] f32 (softmax input)
        ts, tn = transform(a1T[:], maT[:], "w_0_a1", (M, K1))
        a2_nm = sage_combine(ts, tn, inv[:], False, (M, K1), nm_out_dt=F32)

        # ---- softmax over K1 (free dim)
        negmax = small.tile([M, 1], F32, tag="negmax")
        nc.vector.tensor_reduce(out=negmax[:], in_=a2_nm[:], axis=mybir.AxisListType.X,
                                op=AluOp.max, negate=True)
        e_bf = nm.tile([M, K1], BF, tag="e_bf")
        nc.scalar.activation(out=e_bf[:], in_=a2_nm[:], func=Act.Exp,
                             bias=negmax[:], scale=1.0)
        ssum = small.tile([M, 1], F32, tag="ssum")
        nc.vector.tensor_reduce(out=ssum[:], in_=e_bf[:], axis=mybir.AxisListType.X,
                                op=AluOp.add)
        rinv = small.tile([M, 1], F32, tag="rinv")
        nc.vector.reciprocal(out=rinv[:], in_=ssum[:])
        S_nm = nm.tile([M, K1], BF, tag="S_nm")
        nc.vector.tensor_scalar(out=S_nm[:], in0=e_bf[:], scalar1=rinv[:],
                                scalar2=None, op0=AluOp.mult)

        # ---- pooling & graph quantities
        # AS = A @ S (node-major [128, 20])
        as_ps = ps1.tile([128, K1], F32, tag="small")
        nc.tensor.matmul(as_ps[:], lhsT=at_bf[:], rhs=S_nm[:], start=True, stop=True)
        AS_bf = ev.tile([128, K1], BF, tag="AS_bf")
        nc.vector.tensor_copy(out=AS_bf[:], in_=as_ps[:])
        # SAS = S^T (A S)  [20, 20]; SAST = (AS)^T S
        sas_ps = ps1.tile([K1, K1 + K1], F32, tag="small")
        nc.tensor.matmul(sas_ps[:, 0:K1], lhsT=S_nm[:], rhs=AS_bf[:],
                         start=True, stop=True)
        nc.tensor.matmul(sas_ps[:, K1:2 * K1], lhsT=AS_bf[:], rhs=S_nm[:],
                         start=True, stop=True)
        # StS [20, 20]
        sts_ps = ps1.tile([K1, K1], F32, tag="small")
        nc.tensor.matmul(sts_ps[:], lhsT=S_nm[:], rhs=S_nm[:], start=True, stop=True)
        StS = small.tile([K1, K1], F32, tag="StS")
        nc.vector.tensor_copy(out=StS[:], in_=sts_ps[:])
        # pooledT = (S^T h)^T = [F, K1]
        pooled_ps = ps2.tile([F, K1], F32, tag="agg")
        nc.tensor.matmul(pooled_ps[:], lhsT=h_nm[:], rhs=S_nm[:], start=True, stop=True)
        # px^T = relu(pooled + pool_bias) feature-major -> stash into PXT
        pxT = PXT[:, g * 32:g * 32 + K1]
        nc.scalar.activation(out=pxT, in_=pooled_ps[:], func=Act.Relu,
                             bias=poolb[0][:], scale=1.0)
        # graph embedding level 1: max over clusters
        nc.vector.tensor_reduce(out=GH1[:, g:g + 1], in_=pxT,
                                axis=mybir.AxisListType.X, op=AluOp.max)
        # node-major px for level 2: transpose pxT
        lt = g // L2PACK
        lj = g % L2PACK
        pxn_ps = ps1.tile([K1, F], BF, tag="trp")
        nc.tensor.transpose(out=pxn_ps[:], in_=pxT, identity=ident[:])
        nc.scalar.copy(out=PXN[32 * lj:32 * lj + K1, lt * F:(lt + 1) * F],
                       in_=pxn_ps[:])
        # A2T block: SAS^T at rows [32j, 32j+20), cols [tile*128+32j, +20)
        nc.scalar.copy(
            out=A2T[32 * lj:32 * lj + K1,
                    lt * 128 + 32 * lj: lt * 128 + 32 * lj + K1],
            in_=sas_ps[:, K1:2 * K1])

        # ---- level-1 loss scalars
        # num: tr(SAS) -> accum col
        scr = small.tile([K1, K1], BF, tag="scr")
        nc.vector.scalar_tensor_tensor(
            out=scr[:], in0=sas_ps[:, 0:K1], scalar=1.0, in1=ident[0:K1, 0:K1],
            op0=AluOp.mult, op1=AluOp.mult,
            accum_out=SCAL1[0:K1, 0 * gpc + g: 0 * gpc + g + 1])
        # den: sum_n deg * ||S_n||^2
        s2 = small.tile([M, K1], BF, tag="s2")
        s2sum = small.tile([M, 1], F32, tag="s2sum")
        nc.vector.scalar_tensor_tensor(
            out=s2[:], in0=S_nm[:], scalar=1.0, in1=S_nm[:],
            op0=AluOp.mult, op1=AluOp.mult, accum_out=s2sum[:])
        nc.vector.tensor_tensor(
            out=SCAL1[:, 1 * gpc + g:1 * gpc + g + 1], in0=deg[:], in1=s2sum[:],
            op=AluOp.mult)
        # q: sum StS^2 ; t: tr(StS)
        scr2 = small.tile([K1, K1], BF, tag="scr2")
        nc.vector.scalar_tensor_tensor(
            out=scr2[:], in0=StS[:], scalar=1.0, in1=StS[:],
            op0=AluOp.mult, op1=AluOp.mult,
            accum_out=SCAL1[0:K1, 2 * gpc + g:2 * gpc + g + 1])
        nc.vector.scalar_tensor_tensor(
            out=scr[:], in0=StS[:], scalar=1.0, in1=ident[0:K1, 0:K1],
            op0=AluOp.mult, op1=AluOp.mult,
            accum_out=SCAL1[0:K1, 3 * gpc + g:3 * gpc + g + 1])

        if debug and g == 0:
            f32t = ev.tile([M, F], F32, tag="dbg")
            nc.vector.tensor_copy(out=f32t[:], in_=h1_nm[:])
            nc.sync.dma_start(out=dram["dbg_h1"][:, :], in_=f32t[:])
            f32s = ev.tile([M, K1], F32, tag="dbgS")
            nc.vector.tensor_copy(out=f32s[:], in_=S_nm[:])
            nc.sync.dma_start(out=dram["dbg_S"][:, :], in_=f32s[:])
            f32p = ev.tile([F, K1], F32, tag="dbgP")
            nc.vector.tensor_copy(out=f32p[:], in_=pxT)
            nc.sync.dma_start(out=dram["dbg_pxT"][:, :], in_=f32p[:])
            f32a = ev.tile([K1, K1], F32, tag="dbgA")
            nc.vector.tensor_copy(out=f32a[:], in_=sas_ps[:, 0:K1])
            nc.sync.dma_start(out=dram["dbg_SAS"][:, :], in_=f32a[:])

    # ---------------------------------------------------------------- level 2
    P2 = 128           # 4 graphs x (20 live + 12 dead) partitions
    inv2 = 1.0 / float(K1)
    for t in range(nl2):
        a2t = A2T[:, t * 128:(t + 1) * 128]
        pxn_t = PXN[:, t * F:(t + 1) * F]

        # deg2 [128, 1]
        deg2_ps = ps1.tile([P2, 1], F32, tag="small")
        nc.tensor.matmul(deg2_ps[:], lhsT=a2t, rhs=ones_col[:],
                         start=True, stop=True)
        deg2 = small.tile([P2, 1], F32, tag="deg")
        nc.vector.tensor_copy(out=deg2[:], in_=deg2_ps[:])

        # m1' = (A2 px)^T [128, 80]
        m1_ps = ps2.tile([F, P2], F32, tag="agg")
        nc.tensor.matmul(m1_ps[:], lhsT=pxn_t, rhs=a2t, start=True, stop=True)
        m1T2 = fm.tile([F, P2], BF, tag="m1T")
        nc.scalar.copy(out=m1T2[:], in_=m1_ps[:])

        pxT_t = PXT[:, t * 128:(t + 1) * 128]  # [128, 128] fm (32-spaced)

        def transform2(inT, in_mT, wkey, dout_):
            tself = ps1.tile([P2, dout_], F32, tag="tself")
            nc.tensor.matmul(tself[:], lhsT=inT, rhs=W[wkey + "_s"][:],
                             start=True, stop=False)
            nc.tensor.matmul(tself[:], lhsT=ones_row[:],
                             rhs=W["b" + wkey[1:]][:], start=False, stop=True)
            tnei = ps1.tile([P2, dout_], F32, tag="tnei")
            nc.tensor.matmul(tnei[:], lhsT=in_mT, rhs=W[wkey + "_n"][:],
                             start=True, stop=True)
            return tself, tnei

        # feat l1
        ts, tn = transform2(pxT_t, m1T2[:], "w_1_f0", F)
        h1_nm2 = sage_combine(ts, tn, inv2, True, (P2, F))
        h1T2_ps = ps1.tile([F, P2], BF, tag="trp")
        nc.tensor.transpose(out=h1T2_ps[:], in_=h1_nm2[:], identity=ident[:])
        h1T2 = fm.tile([F, P2], BF, tag="h1T")
        nc.scalar.copy(out=h1T2[:], in_=h1T2_ps[:])
        m2_ps2 = ps2.tile([F, P2], F32, tag="agg")
        nc.tensor.matmul(m2_ps2[:], lhsT=h1_nm2[:], rhs=a2t, start=True, stop=True)
        m2T2 = fm.tile([F, P2], BF, tag="m2T")
        nc.scalar.copy(out=m2T2[:], in_=m2_ps2[:])
        ts, tn = transform2(h1T2[:], m2T2[:], "w_1_f1", F)
        h_nm2 = sage_combine(ts, tn, inv2, False, (P2, F))

        # assign
        ts, tn = transform2(pxT_t, m1T2[:], "w_1_a0", F)
        a1_nm2 = sage_combine(ts, tn, inv2, True, (P2, F))
        a1T2_ps = ps1.tile([F, P2], BF, tag="trp")
        nc.tensor.transpose(out=a1T2_ps[:], in_=a1_nm2[:], identity=ident[:])
        a1T2 = fm.tile([F, P2], BF, tag="a1T")
        nc.scalar.copy(out=a1T2[:], in_=a1T2_ps[:])
        ma_ps2 = ps2.tile([F, P2], F32, tag="agg")
        nc.tensor.matmul(ma_ps2[:], lhsT=a1_nm2[:], rhs=a2t, start=True, stop=True)
        maT2 = fm.tile([F, P2], BF, tag="m2T")
        nc.scalar.copy(out=maT2[:], in_=ma_ps2[:])
        ts, tn = transform2(a1T2[:], maT2[:], "w_1_a1", K2)
        a2_nm2 = sage_combine(ts, tn, inv2, False, (P2, K2), nm_out_dt=F32)

        # softmax over K2
        negmax2 = small.tile([P2, 1], F32, tag="negmax")
        nc.vector.tensor_reduce(out=negmax2[:], in_=a2_nm2[:],
                                axis=mybir.AxisListType.X, op=AluOp.max, negate=True)
        e2 = nm.tile([P2, K2], BF, tag="e_bf")
        nc.scalar.activation(out=e2[:], in_=a2_nm2[:], func=Act.Exp,
                             bias=negmax2[:], scale=1.0)
        ssum2 = small.tile([P2, 1], F32, tag="ssum")
        nc.vector.tensor_reduce(out=ssum2[:], in_=e2[:], axis=mybir.AxisListType.X,
                                op=AluOp.add)
        rinv2 = small.tile([P2, 1], F32, tag="rinv")
        nc.vector.reciprocal(out=rinv2[:], in_=ssum2[:])
        S2_nm = nm.tile([P2, K2], BF, tag="S_nm")
        nc.vector.tensor_scalar(out=S2_nm[:], in0=e2[:], scalar1=rinv2[:],
                                scalar2=None, op0=AluOp.mult)
        # expand to block layout [80, 20]
        S2e = nm.tile([P2, L2PACK * K2], BF, tag="S2e")
        base = S2_nm[:]
        s2rep = bass.AP(tensor=base.tensor, offset=base.offset,
                        ap=[base.ap[0], [0, L2PACK], base.ap[1]])
        nc.vector.tensor_tensor(out=S2e[:], in0=s2rep, in1=bmask[0:P2, :],
                                op=AluOp.mult)

        # AS2 = A2 @ S2e [80, 20]
        as2_ps = ps1.tile([P2, L2PACK * K2], F32, tag="small")
        nc.tensor.matmul(as2_ps[:], lhsT=a2t, rhs=S2e[:], start=True, stop=True)
        AS2 = ev.tile([P2, L2PACK * K2], BF, tag="AS_bf")
        nc.vector.tensor_copy(out=AS2[:], in_=as2_ps[:])
        sas2_ps = ps1.tile([L2PACK * K2, L2PACK * K2], F32, tag="small")
        nc.tensor.matmul(sas2_ps[:], lhsT=S2e[:], rhs=AS2[:], start=True, stop=True)
        sts2_ps = ps1.tile([L2PACK * K2, L2PACK * K2], F32, tag="small")
        nc.tensor.matmul(sts2_ps[:], lhsT=S2e[:], rhs=S2e[:], start=True, stop=True)
        StS2 = small.tile([L2PACK * K2, L2PACK * K2], F32, tag="StS")
        nc.vector.tensor_copy(out=StS2[:], in_=sts2_ps[:])
        # pooled2T [128, 20]
        pooled2_ps = ps2.tile([F, L2PACK * K2], F32, tag="agg")
        nc.tensor.matmul(pooled2_ps[:], lhsT=h_nm2[:], rhs=S2e[:],
                         start=True, stop=True)
        px2T = ev.tile([F, L2PACK * K2], BF, tag="px2T")
        nc.scalar.activation(out=px2T[:], in_=pooled2_ps[:], func=Act.Relu,
                             bias=poolb[1][:], scale=1.0)
        # graph embedding level 2: max over each graph's 5 clusters
        px2g = px2T[:].rearrange("f (j k) -> f j k", j=L2PACK)
        nc.vector.tensor_reduce(out=GH2[:, t * L2PACK:(t + 1) * L2PACK],
                                in_=px2g, axis=mybir.AxisListType.X, op=AluOp.max)

        # ---- level-2 loss scalars
        d20 = L2PACK * K2
        scr3 = small.tile([d20, d20], BF, tag="scr")
        nc.vector.scalar_tensor_tensor(
            out=scr3[:], in0=sas2_ps[:], scalar=1.0, in1=ident[0:d20, 0:d20],
            op0=AluOp.mult, op1=AluOp.mult,
            accum_out=SCAL2a[0:d20, 0 * nl2 + t:0 * nl2 + t + 1])
        s22 = small.tile([P2, K2], BF, tag="s2")
        s2sum2 = small.tile([P2, 1], F32, tag="s2sum")
        nc.vector.scalar_tensor_tensor(
            out=s22[:], in0=S2_nm[:], scalar=1.0, in1=S2_nm[:],
            op0=AluOp.mult, op1=AluOp.mult, accum_out=s2sum2[:])
        nc.vector.tensor_tensor(
            out=SCAL2b[:, t:t + 1], in0=deg2[:], in1=s2sum2[:], op=AluOp.mult)
        scr4 = small.tile([d20, d20], BF, tag="scr2")
        nc.vector.scalar_tensor_tensor(
            out=scr4[:], in0=StS2[:], scalar=1.0, in1=StS2[:],
            op0=AluOp.mult, op1=AluOp.mult,
            accum_out=SCAL2a[0:d20, 1 * nl2 + t:1 * nl2 + t + 1])
        nc.vector.scalar_tensor_tensor(
            out=scr3[:], in0=StS2[:], scalar=1.0, in1=ident[0:d20, 0:d20],
            op0=AluOp.mult, op1=AluOp.mult,
            accum_out=SCAL2a[0:d20, 2 * nl2 + t:2 * nl2 + t + 1])

    # ------------------------------------------------------------- reductions
    # SUMS1 [1, 4*gpc] ; SUMS2a [4, 3*nl2] ; SUMS2b [4, nl2]
    sums1_ps = ps1.tile([1, 4 * gpc], F32, tag="small")
    nc.tensor.matmul(sums1_ps[:], lhsT=ones_col_f[:], rhs=SCAL1[:],
                     start=True, stop=True)
    S1 = small.tile([1, 4 * gpc], F32, tag="S1")
    nc.vector.tensor_copy(out=S1[:], in_=sums1_ps[:])
    sums2a_ps = ps1.tile([L2PACK, 3 * nl2], F32, tag="small")
    nc.tensor.matmul(sums2a_ps[:], lhsT=blk5[:], rhs=SCAL2a[0:L2PACK * K2, :],
                     start=True, stop=True)
    S2a = small.tile([L2PACK, 3 * nl2], F32, tag="S2a")
    nc.vector.tensor_copy(out=S2a[:], in_=sums2a_ps[:])
    sums2b_ps = ps1.tile([L2PACK, nl2], F32, tag="small")
    nc.tensor.matmul(sums2b_ps[:], lhsT=blk20[:], rhs=SCAL2b[:],
                     start=True, stop=True)
    S2b = small.tile([L2PACK, nl2], F32, tag="S2b")
    nc.vector.tensor_copy(out=S2b[:], in_=sums2b_ps[:])

    if debug:
        nc.sync.dma_start(out=dram["dbg_scal"][:, :], in_=SCAL1[:])

    def loss_block(num_ap, den_ap, q_ap, t_ap, dims, kk):
        """Returns (cut [p,n], orth [p,n]) tiles for per-graph scalars."""
        p, n = dims
        dn = small.tile([p, n], F32, tag="dn")
        nc.vector.tensor_scalar(out=dn[:], in0=den_ap, scalar1=EPS, scalar2=None,
                                op0=AluOp.add)
        rd = small.tile([p, n], F32, tag="rd")
        nc.vector.reciprocal(out=rd[:], in_=dn[:])
        cut = small.tile([p, n], F32, tag="cut")
        nc.vector.tensor_tensor(out=cut[:], in0=num_ap, in1=rd[:], op=AluOp.mult)
        qe = small.tile([p, n], F32, tag="qe")
        nc.vector.tensor_scalar(out=qe[:], in0=q_ap, scalar1=EPS, scalar2=None,
                                op0=AluOp.add)
        rq = small.tile([p, n], F32, tag="rq")
        nc.vector.reciprocal(out=rq[:], in_=qe[:])
        t1 = small.tile([p, n], F32, tag="t1")
        nc.vector.tensor_tensor(out=t1[:], in0=q_ap, in1=rq[:], op=AluOp.mult)
        fro = small.tile([p, n], F32, tag="fro")
        nc.scalar.activation(out=fro[:], in_=qe[:], func=Act.Sqrt, bias=0.0, scale=1.0)
        rs = small.tile([p, n], F32, tag="rs")
        nc.vector.reciprocal(out=rs[:], in_=fro[:])
        t2 = small.tile([p, n], F32, tag="t2")
        nc.vector.scalar_tensor_tensor(
            out=t2[:], in0=t_ap, scalar=-2.0 / float(np.sqrt(kk)), in1=rs[:],
            op0=AluOp.mult, op1=AluOp.mult)
        s = small.tile([p, n], F32, tag="s")
        nc.vector.scalar_tensor_tensor(
            out=s[:], in0=t1[:], scalar=1.0 + EPS, in1=t2[:],
            op0=AluOp.add, op1=AluOp.add)
        orth = small.tile([p, n], F32, tag="orth")
        nc.scalar.activation(out=orth[:], in_=s[:], func=Act.Sqrt, bias=0.0, scale=1.0)
        return cut, orth

    cut1, orth1 = loss_block(S1[:, 0:gpc], S1[:, gpc:2 * gpc],
                             S1[:, 2 * gpc:3 * gpc], S1[:, 3 * gpc:4 * gpc],
                             (1, gpc), K1)
    cut2, orth2 = loss_block(S2a[:, 0:nl2], S2b[:, 0:nl2],
                             S2a[:, nl2:2 * nl2], S2a[:, 2 * nl2:3 * nl2],
                             (L2PACK, nl2), K2)

    # reduce to totals: row sums then (for level2) partition sums via matmul
    red1 = small.tile([1, 2], F32, tag="red1")
    nc.vector.tensor_reduce(out=red1[:, 0:1], in_=cut1[:], axis=mybir.AxisListType.X,
                            op=AluOp.add)
    nc.vector.tensor_reduce(out=red1[:, 1:2], in_=orth1[:], axis=mybir.AxisListType.X,
                            op=AluOp.add)
    red2col = small.tile([L2PACK, 2], F32, tag="red2col")
    nc.vector.tensor_reduce(out=red2col[:, 0:1], in_=cut2[:],
                            axis=mybir.AxisListType.X, op=AluOp.add)
    nc.vector.tensor_reduce(out=red2col[:, 1:2], in_=orth2[:],
                            axis=mybir.AxisListType.X, op=AluOp.add)
    red2_ps = ps1.tile([1, 2], F32, tag="small")
    nc.tensor.matmul(red2_ps[:], lhsT=ones_col_f[0:L2PACK, :], rhs=red2col[:],
                     start=True, stop=True)
    total = small.tile([1, 2], F32, tag="total")
    nc.vector.tensor_tensor(out=total[:], in0=red1[:], in1=red2_ps[:], op=AluOp.add)
    # negate cut (cut_g = -num/den; device computed +num/den)
    lout = small.tile([1, 2], F32, tag="lout")
    nc.vector.tensor_scalar(out=lout[:, 0:1], in0=total[:, 0:1], scalar1=-1.0,
                            scalar2=None, op0=AluOp.mult)
    nc.vector.tensor_copy(out=lout[:, 1:2], in_=total[:, 1:2])
    nc.sync.dma_start(out=dram["losses"][:, :], in_=lout[:])

    # ------------------------------------------------------------- logits
    logits_ps = ps1.tile([2, gpc], F32, tag="small")
    ones_row_f = singles.tile([1, gpc], F32)
    nc.vector.memset(ones_row_f[:], 1.0)
    nc.tensor.matmul(logits_ps[:], lhsT=mlp_w1[:], rhs=GH1[:], start=True, stop=False)
    nc.tensor.matmul(logits_ps[:], lhsT=mlp_w2[:], rhs=GH2[:], start=False, stop=False)
    nc.tensor.matmul(logits_ps[:], lhsT=mlp_b[:], rhs=ones_row_f[:],
                     start=False, stop=True)
    logits_sb = small.tile([2, gpc], F32, tag="logits_sb")
    nc.vector.tensor_copy(out=logits_sb[:], in_=logits_ps[:])
    nc.sync.dma_start(out=dram["logitsT"][:, :], in_=logits_sb[:])

    ctx.close()


# ------------------------------------------------------------------ runtime
def _ensure_axon_hooks():
    """bass_utils imports antenv.axon_hooks when BASS_TRACE is set; some
    images lack that module.  Shim it (with the real ctypes NTFF hook when
    the boot package provides one) so tracing works / degrades gracefully."""
    import sys, types
    try:
        import antenv.axon_hooks  # noqa: F401
        return
    except ImportError:
        pass
    mod = types.ModuleType("antenv.axon_hooks")
    _h = [None]
    mod.set_axon_ntff_profile_hook = lambda h: _h.__setitem__(0, h)
    mod.get_axon_ntff_profile_hook = lambda: _h[0]
    sys.modules["antenv.axon_hooks"] = mod
    try:
        from trn_agent_boot.trn_boot import _ntff_profile_via_ctypes
        hook = _ntff_profile_via_ctypes("/opt/axon/libaxon_pjrt.so")
        if hook is not None:
            mod.set_axon_ntff_profile_hook(hook)
    except Exception:
        pass


_ensure_axon_hooks()

_CACHE = {}
LAST_RESULTS = None


def _get_program(gpc=GPC, debug=False):
    key = (gpc, debug)
    if key not in _CACHE:
        _CACHE[key] = build_program(gpc, debug)
    return _CACHE[key]


def kernel(x, edge_index, edge_weight, node_graph_index, params):
    x = np.asarray(x)
    edge_index = np.asarray(edge_index)
    edge_weight = np.asarray(edge_weight)

    nc, dram = _get_program()
    wmap = pack_params(params)

    in_maps = []
    for core in range(NCORES):
        m = shard_inputs(x, edge_index, edge_weight, params, core)
        m.update(wmap)
        in_maps.append(m)

    res = bass_utils.run_bass_kernel_spmd(nc, in_maps, core_ids=list(range(NCORES)))
    global LAST_RESULTS
    LAST_RESULTS = res

    logits = np.concatenate([r["logitsT"].T for r in res.results], 0)
    cut = sum(float(r["losses"][0, 0]) for r in res.results) / NUM_GRAPHS
    orth = sum(float(r["losses"][0, 1]) for r in res.results) / NUM_GRAPHS
    return (np.asarray(logits, np.float32),
            np.float32(cut), np.float32(orth))
